# revision 10
# baseline (speedup 1.0000x reference)
"""Trainium2 Bass kernel for nn_CCL_50740743635433 (class-collapsed CCL loss).

Math: with C=64 classes, pos_centroid[i] == class_centroid[labels[i]], so the
reference's 8192x8192 distance matrix collapses to 8192x64:
  class_sum[c,:]  = sum_{i: lab_i==c} preds[i,:]      (one-hot matmul)
  cent[c,:]       = class_sum[c,:] / count[c]
  sq[i,c]         = relu(|p_i|^2 + |cent_c|^2 - 2 p_i.cent_c)
  pos[i]          = sqrt(sq[i, lab_i]);  neg[i] = sqrt(min_{c != lab_i} sq[i,c])
  loss            = mean softplus(pos - neg + 0.2)

Distribution (8 cores, no collectives): every core receives the FULL preds and
computes the class sums redundantly (cross-core collectives cost ~70us on this
rig vs ~10us of local compute); each core then evaluates distances + softplus
only for its own 1024-row shard and returns a partial sum; the host adds the 8
partials and divides by N.

Precision: the two big matmuls run in bf16 (fp32 matmul is 4 cyc/row on the PE,
bf16 is 1). Verified numerically: bf16 inputs to the class-sum and Gram matmuls
move the final loss by ~2e-7 relative (errors wash out in the 8192-row mean).
Everything else (p^2, c^2, centroid division, masked min, softplus) is fp32.
sqrt is computed on the vector engine via Newton rsqrt so the scalar engine
stays on a single activation table (Copy/Square/Relu/Exp/Ln) - a table switch
costs ~2.7us.
"""

import sys

sys.path.insert(0, "/opt/trn_rl_repo")

import numpy as np

import concourse.bacc as bacc
import concourse.bass_utils as bass_utils
import concourse.mybir as mybir
import concourse.tile as tile

N = 8192
D = 128
C = 64
N_CORES = 8
ROWS_PER_CORE = N // N_CORES          # 1024
CHUNKS = N // 128                     # 64 chunks of 128 rows
OWN_CHUNKS = ROWS_PER_CORE // 128     # 8 chunks per core
ALPHA = 0.2
BIG = 1e10
HUGE = 1e20

f32 = mybir.dt.float32
bf16 = mybir.dt.bfloat16
i32 = mybir.dt.int32
Alu = mybir.AluOpType
Act = mybir.ActivationFunctionType
Ax = mybir.AxisListType

_compiled = None
last_results = None


def _build():
    nc = bacc.Bacc(
        "TRN2",
        target_bir_lowering=False,
        debug=False,
        enable_asserts=True,
        num_devices=N_CORES,
    )

    preds_d = nc.dram_tensor("preds", [N, D], f32, kind="ExternalInput")
    labels_d = nc.dram_tensor("labels", [128, CHUNKS], f32, kind="ExternalInput")
    mypreds_d = nc.dram_tensor("my_preds", [ROWS_PER_CORE, D], f32, kind="ExternalInput")
    mylab_d = nc.dram_tensor("my_labels", [128, OWN_CHUNKS], f32, kind="ExternalInput")
    out_d = nc.dram_tensor("out", [1, 1], f32, kind="ExternalOutput")

    iota_d = nc.inline_tensor(
        np.tile(np.arange(C, dtype=np.float32), (128, 1)), name="iota64"
    )
    ident_d = nc.inline_tensor(np.eye(128, dtype=np.float32), name="ident128")
    import ml_dtypes

    identb_d = nc.inline_tensor(
        np.eye(128, dtype=ml_dtypes.bfloat16), name="identb128"
    )
    onesc_d = nc.inline_tensor(np.ones((128, 1), dtype=np.float32), name="ones_col")
    onesr_d = nc.inline_tensor(np.ones((1, 128), dtype=np.float32), name="ones_row")

    with tile.TileContext(nc) as tc:
        with (
            tc.tile_pool(name="cst", bufs=1) as cst,
            tc.tile_pool(name="big", bufs=1) as bigp,
            tc.tile_pool(name="wrk", bufs=1) as wrk,
            tc.tile_pool(name="scr", bufs=2) as scr,
            tc.tile_pool(name="pacc", bufs=1, space="PSUM") as pacc,
            tc.tile_pool(name="pt", bufs=2, space="PSUM") as pt,
            tc.tile_pool(name="pg", bufs=2, space="PSUM") as pg,
            tc.tile_pool(name="psm", bufs=2, space="PSUM") as psm,
        ):
            # ---- constants / inputs to SBUF ----
            iota_sb = cst.tile([128, C], f32)
            nc.sync.dma_start(iota_sb[:], iota_d.ap())
            ident_sb = cst.tile([128, 128], f32)
            nc.sync.dma_start(ident_sb[:], ident_d.ap())
            identb_sb = cst.tile([128, 128], bf16)
            nc.sync.dma_start(identb_sb[:], identb_d.ap())
            onesc_sb = cst.tile([128, 1], f32)
            nc.sync.dma_start(onesc_sb[:], onesc_d.ap())
            onesr_sb = cst.tile([1, 128], f32)
            nc.sync.dma_start(onesr_sb[:], onesr_d.ap())
            alpha_sb = cst.tile([128, 1], f32)
            nc.vector.memset(alpha_sb[:], ALPHA)

            lsb = cst.tile([128, CHUNKS], f32)
            nc.sync.dma_start(lsb[:], labels_d.ap())
            mylsb = cst.tile([128, OWN_CHUNKS], f32)
            nc.sync.dma_start(mylsb[:], mylab_d.ap())

            # one-hot for ALL chunks in one op (broadcast APs):
            # oh_all[p, j, c] = (lsb[p, j] == iota[p, c]), bf16
            oh_all = bigp.tile([128, CHUNKS, C], bf16)
            nc.vector.tensor_tensor(
                oh_all[:],
                lsb[:].to_broadcast((128, CHUNKS, C)),
                iota_sb[:].rearrange("p (j c) -> p j c", j=1).to_broadcast(
                    (128, CHUNKS, C)
                ),
                Alu.is_equal,
            )
            # own-chunk masks (fp32): ohR = 1e10*onehot, invR = 1e10*(1-onehot)
            mk = wrk.tile([128, OWN_CHUNKS, C], f32)
            nc.vector.tensor_tensor(
                mk[:],
                mylsb[:].to_broadcast((128, OWN_CHUNKS, C)),
                iota_sb[:].rearrange("p (j c) -> p j c", j=1).to_broadcast(
                    (128, OWN_CHUNKS, C)
                ),
                Alu.is_equal,
            )
            ohR = wrk.tile([128, OWN_CHUNKS, C], f32)
            nc.vector.tensor_scalar(ohR[:], mk[:], BIG, None, Alu.mult)
            invR = wrk.tile([128, OWN_CHUNKS, C], f32)
            nc.vector.tensor_scalar(invR[:], mk[:], -BIG, BIG, Alu.mult, Alu.add)

            # full preds (fp32 staging + bf16 copy with an appended ones col)
            psb = bigp.tile([128, CHUNKS, D], f32)
            psb_bf = bigp.tile([128, CHUNKS, D + 1], bf16)
            nc.vector.memset(psb_bf[:, :, D : D + 1], 1.0)
            preds_re = preds_d.ap().rearrange("(j p) d -> p j d", p=128)
            GROUPS = 8
            G = CHUNKS // GROUPS
            for g in range(GROUPS):
                eng = nc.sync if g % 2 == 0 else nc.scalar
                eng.dma_start(
                    psb[:, g * G : (g + 1) * G, :],
                    preds_re[:, g * G : (g + 1) * G, :],
                )
                nc.gpsimd.tensor_copy(
                    psb_bf[:, g * G : (g + 1) * G, 0:D],
                    psb[:, g * G : (g + 1) * G, :],
                )

            # own shard, chunk-major, fp32 + bf16
            osb = wrk.tile([128, OWN_CHUNKS, D], f32)
            nc.scalar.dma_start(
                osb[:], mypreds_d.ap().rearrange("(j p) d -> p j d", p=128)
            )
            osb_bf = wrk.tile([128, OWN_CHUNKS, D], bf16)
            nc.vector.tensor_copy(osb_bf[:], osb[:])

            # ---- phase A: class sums + counts via one-hot matmuls (bf16) ----
            # psum_cs[c, 0:D] = sum_i oh[i,c]*preds[i,:], [c, D] = count[c]
            psum_cs = pacc.tile([C, D + 1], f32)
            for j in range(CHUNKS):
                nc.tensor.matmul(
                    psum_cs[:],
                    oh_all[:, j, :],
                    psb_bf[:, j, :],
                    start=(j == 0),
                    stop=(j == CHUNKS - 1),
                )

            # ---- own-shard prep (independent of phase A results) ----
            psq = wrk.tile([128, OWN_CHUNKS], f32)
            pts_bf = wrk.tile([128, OWN_CHUNKS, D], bf16)
            for j in range(OWN_CHUNKS):
                sqscr = scr.tile([128, D], f32, name=f"sqscr{j}", tag="sqscr")
                nc.scalar.activation(
                    sqscr[:], osb[:, j, :], Act.Square,
                    accum_out=psq[:, j : j + 1],
                )
                ptb = pt.tile([128, 128], bf16, name=f"ptb{j}", tag="ptb")
                nc.tensor.transpose(ptb[:], osb_bf[:, j, :], identb_sb[:])
                nc.scalar.activation(pts_bf[:, j, :], ptb[:], Act.Copy, scale=-2.0)

            # ---- centroids ----
            cs_sb = wrk.tile([C, D + 1], f32)
            nc.scalar.activation(cs_sb[:], psum_cs[:], Act.Copy)
            # rcat cols: [1/max(cnt,1) | 1e20*(cnt==0)] (column space, base 0)
            rcat = wrk.tile([C, 2], f32)
            safe = wrk.tile([C, 1], f32)
            nc.vector.tensor_scalar(
                safe[:], cs_sb[:, D : D + 1], 1.0, None, Alu.max
            )
            nc.vector.reciprocal(rcat[:, 0:1], safe[:])
            nc.vector.tensor_scalar(
                rcat[:, 1:2], cs_sb[:, D : D + 1], 0.0, HUGE,
                Alu.is_equal, Alu.mult,
            )
            # transpose each column to a row (both land at partition 0)
            psum_rt = psm.tile([1, C], f32, name="psum_rt", tag="sm")
            nc.tensor.matmul(psum_rt[:], rcat[:, 0:1], ident_sb[0:C, 0:C])
            rrow = wrk.tile([1, C], f32)
            nc.scalar.activation(rrow[:], psum_rt[:], Act.Copy)
            psum_ab = psm.tile([1, C], f32, name="psum_ab", tag="sm")
            nc.tensor.matmul(psum_ab[:], rcat[:, 1:2], ident_sb[0:C, 0:C])
            ab_sb = wrk.tile([1, C], f32)
            nc.scalar.activation(ab_sb[:], psum_ab[:], Act.Copy)

            # centT[d, c] = class_sum[c, d] * recip[c]  (fp32)
            psum_ct = pt.tile([128, C], f32, name="psum_ct", tag="ctp", bufs=1)
            nc.tensor.transpose(psum_ct[:], cs_sb[:, 0:D], ident_sb[0:C, 0:C])
            ctsb = wrk.tile([128, C], f32)
            nc.scalar.activation(ctsb[:], psum_ct[:], Act.Copy)
            psum_rb = psm.tile([128, C], f32, name="psum_rb", tag="sm")
            nc.tensor.matmul(psum_rb[:], onesr_sb[:], rrow[:])
            centT = wrk.tile([128, C], f32)
            nc.vector.tensor_tensor(centT[:], ctsb[:], psum_rb[:], Alu.mult)
            centT_bf = wrk.tile([128, C], bf16)
            nc.vector.tensor_copy(centT_bf[:], centT[:])

            # c_sq row (+1e20 on absent classes), broadcast to all partitions
            sqc = wrk.tile([128, C], f32)
            nc.vector.tensor_tensor(sqc[:], centT[:], centT[:], Alu.mult)
            psum_csq = psm.tile([1, C], f32, name="psum_csq", tag="sm")
            nc.tensor.matmul(psum_csq[:], onesc_sb[:], sqc[:])
            csqr = wrk.tile([1, C], f32)
            nc.vector.tensor_tensor(csqr[:], psum_csq[:], ab_sb[:], Alu.add)
            psum_cb = psm.tile([128, C], f32, name="psum_cb", tag="sm")
            nc.tensor.matmul(psum_cb[:], onesr_sb[:], csqr[:])
            csq_sb = wrk.tile([128, C], f32)
            nc.scalar.activation(csq_sb[:], psum_cb[:], Act.Copy)

            # ---- phase F: per own chunk distances, masked mins ----
            # pnsq even cols = negsq (min over other classes), odd = possq
            pnsq = wrk.tile([128, 2 * OWN_CHUNKS], f32)
            for j in range(OWN_CHUNKS):
                psum_g = pg.tile([128, C], f32, name=f"psum_g{j}", tag="g")
                nc.tensor.matmul(psum_g[:], pts_bf[:, j, :], centT_bf[:])
                hc = scr.tile([128, C], f32, name=f"hc{j}", tag="hc")
                nc.vector.tensor_tensor(hc[:], psum_g[:], csq_sb[:], Alu.add)
                sqj = scr.tile([128, C], f32, name=f"sqj{j}", tag="sqj")
                nc.scalar.activation(
                    sqj[:], hc[:], Act.Relu, bias=psq[:, j : j + 1]
                )
                pair = scr.tile([128, 2, C], f32, name=f"pair{j}", tag="pair")
                nc.vector.tensor_tensor(
                    pair[:, 0, :], sqj[:], ohR[:, j, :], Alu.add
                )
                nc.vector.tensor_tensor(
                    pair[:, 1, :], sqj[:], invR[:, j, :], Alu.add
                )
                nc.vector.tensor_reduce(
                    pnsq[:, 2 * j : 2 * j + 2], pair[:], Ax.X, Alu.min
                )

            # ---- tail: sqrt via Newton rsqrt on DVE, then softplus ----
            W = 2 * OWN_CHUNKS
            z = wrk.tile([128, W], f32)
            tsh = wrk.tile([128, W], f32)
            nc.vector.tensor_scalar(
                tsh[:].bitcast(i32), pnsq[:].bitcast(i32), 1, None,
                Alu.logical_shift_right,
            )
            nc.vector.tensor_scalar(
                z[:].bitcast(i32), tsh[:].bitcast(i32), -1, 0x5F3759DF,
                Alu.mult, Alu.add,
            )
            t1 = wrk.tile([128, W], f32)
            for _ in range(3):
                nc.vector.tensor_tensor(t1[:], z[:], z[:], Alu.mult)
                nc.vector.tensor_tensor(t1[:], t1[:], pnsq[:], Alu.mult)
                nc.vector.tensor_scalar(
                    t1[:], t1[:], -0.5, 1.5, Alu.mult, Alu.add
                )
                nc.vector.tensor_tensor(z[:], z[:], t1[:], Alu.mult)
            pn = wrk.tile([128, W], f32)
            nc.vector.tensor_tensor(pn[:], pnsq[:], z[:], Alu.mult)

            # softplus(pos - neg + alpha) = ln(1 + exp(...))
            x = wrk.tile([128, OWN_CHUNKS], f32)
            nc.vector.tensor_tensor(
                x[:], pn[:, 1::2], pn[:, 0::2], Alu.subtract
            )
            e = wrk.tile([128, OWN_CHUNKS], f32)
            nc.scalar.activation(e[:], x[:], Act.Exp, bias=alpha_sb[:])
            sp = wrk.tile([128, OWN_CHUNKS], f32)
            nc.scalar.activation(sp[:], e[:], Act.Ln, bias=1.0)
            rowsum = wrk.tile([128, 1], f32)
            nc.vector.tensor_reduce(rowsum[:], sp[:], Ax.X, Alu.add)
            psum_out = psm.tile([1, 1], f32, name="psum_out", tag="sm")
            nc.tensor.matmul(psum_out[:], rowsum[:], onesc_sb[:])
            out_sb = wrk.tile([1, 1], f32)
            nc.scalar.activation(out_sb[:], psum_out[:], Act.Copy)
            nc.sync.dma_start(out_d.ap(), out_sb[:])

    nc.compile()
    return nc


def _get_compiled():
    global _compiled
    if _compiled is None:
        _compiled = _build()
    return _compiled


def _chunk_major_labels(lab_f32):
    # labels[j*128 + p] -> [p, j]
    n_chunks = lab_f32.shape[0] // 128
    return np.ascontiguousarray(lab_f32.reshape(n_chunks, 128).T)


def kernel(preds, labels, _trace=False):
    preds = np.ascontiguousarray(np.asarray(preds, dtype=np.float32))
    lab_f32 = np.asarray(labels, dtype=np.float32)
    assert preds.shape == (N, D) and lab_f32.shape == (N,)

    nc = _get_compiled()
    lab_cm = _chunk_major_labels(lab_f32)
    in_maps = []
    for c in range(N_CORES):
        r0, r1 = c * ROWS_PER_CORE, (c + 1) * ROWS_PER_CORE
        in_maps.append(
            {
                "preds": preds,
                "labels": lab_cm,
                "my_preds": np.ascontiguousarray(preds[r0:r1]),
                "my_labels": _chunk_major_labels(lab_f32[r0:r1]),
            }
        )

    res = bass_utils.run_bass_kernel_spmd(
        nc, in_maps, core_ids=list(range(N_CORES)), trace=_trace
    )
    global last_results
    last_results = res
    total = sum(float(res.results[c]["out"][0, 0]) for c in range(N_CORES))
    return np.float32(total / N)


# revision 12
# speedup vs baseline: 1.3234x; 1.3234x over previous
"""Trainium2 Bass kernel for nn_CCL_50740743635433 (class-collapsed CCL loss).

Math: with C=64 classes, pos_centroid[i] == class_centroid[labels[i]], so the
reference's 8192x8192 distance matrix collapses to 8192x64:
  class_sum[c,:]  = sum_{i: lab_i==c} preds[i,:]      (one-hot matmul)
  cent[c,:]       = class_sum[c,:] / count[c]
  sq[i,c]         = relu(|p_i|^2 + |cent_c|^2 - 2 p_i.cent_c)
  pos[i]          = sqrt(sq[i, lab_i]);  neg[i] = sqrt(min_{c != lab_i} sq[i,c])
  loss            = mean softplus(pos - neg + 0.2)

Distribution (8 cores, no collectives): every core receives the FULL preds and
computes the class sums redundantly (cross-core collectives cost ~70us on this
rig vs ~15us of local compute); each core then evaluates distances + softplus
only for its own 1024-row shard and returns a partial sum; the host adds the 8
partials and divides by N.

Perf structure:
- The two big matmuls run in bf16 (fp32 matmul is 4 cyc/row on the PE; the
  numerical effect on the final loss is ~2e-7 relative - errors wash out in
  the 8192-row mean). Everything else is fp32.
- Phase A packs even/odd chunks into the two 64-column halves of the PE array
  (tile_position) so two matmuls run concurrently and their LDWEIGHTS overlap;
  the two half-sums are merged along the free axis after a PE transpose.
- sqrt is computed on the vector engine via Newton rsqrt so the scalar engine
  stays on one activation table (Copy/Square/Relu/Exp/Ln); a switch is ~2.7us.
- f32->bf16 casts run on the vector engine (measured ~0.7us/128x1024; the
  GPSIMD path is ~6x slower), pipelined per DMA group.
"""

import sys

sys.path.insert(0, "/opt/trn_rl_repo")

import numpy as np

import concourse.bacc as bacc
import concourse.bass_utils as bass_utils
import concourse.mybir as mybir
import concourse.tile as tile

N = 8192
D = 128
C = 64
N_CORES = 8
ROWS_PER_CORE = N // N_CORES          # 1024
CHUNKS = N // 128                     # 64 chunks of 128 rows
OWN_CHUNKS = ROWS_PER_CORE // 128     # 8 chunks per core
ALPHA = 0.2
BIG = 1e10
HUGE = 1e20

f32 = mybir.dt.float32
bf16 = mybir.dt.bfloat16
i32 = mybir.dt.int32
Alu = mybir.AluOpType
Act = mybir.ActivationFunctionType
Ax = mybir.AxisListType

_compiled = None
last_results = None


def _build():
    import ml_dtypes

    nc = bacc.Bacc(
        "TRN2",
        target_bir_lowering=False,
        debug=False,
        enable_asserts=True,
        num_devices=N_CORES,
    )

    preds_d = nc.dram_tensor("preds", [N, D], f32, kind="ExternalInput")
    labels_d = nc.dram_tensor("labels", [128, CHUNKS], f32, kind="ExternalInput")
    mypreds_d = nc.dram_tensor("my_preds", [ROWS_PER_CORE, D], f32, kind="ExternalInput")
    mylab_d = nc.dram_tensor("my_labels", [128, OWN_CHUNKS], f32, kind="ExternalInput")
    out_d = nc.dram_tensor("out", [1, 1], f32, kind="ExternalOutput")

    iota_d = nc.inline_tensor(
        np.tile(np.arange(C, dtype=np.float32), (128, 1)), name="iota64"
    )
    ident_d = nc.inline_tensor(np.eye(128, dtype=np.float32), name="ident128")
    identb_d = nc.inline_tensor(
        np.eye(128, dtype=ml_dtypes.bfloat16), name="identb128"
    )
    onesc_d = nc.inline_tensor(np.ones((128, 1), dtype=np.float32), name="ones_col")
    onesr_d = nc.inline_tensor(np.ones((1, 128), dtype=np.float32), name="ones_row")

    with tile.TileContext(nc) as tc:
        with (
            tc.tile_pool(name="cst", bufs=1) as cst,
            tc.tile_pool(name="big", bufs=1) as bigp,
            tc.tile_pool(name="wrk", bufs=1) as wrk,
            tc.tile_pool(name="scr", bufs=2) as scr,
            tc.tile_pool(name="pacc", bufs=1, space="PSUM") as pacc,
            tc.tile_pool(name="pt", bufs=2, space="PSUM") as pt,
            tc.tile_pool(name="pg", bufs=2, space="PSUM") as pg,
            tc.tile_pool(name="psm", bufs=2, space="PSUM") as psm,
        ):
            # ---- constants / small inputs first (tiny DMAs on sync) ----
            lsb = cst.tile([128, CHUNKS], f32)
            nc.sync.dma_start(lsb[:], labels_d.ap())
            mylsb = cst.tile([128, OWN_CHUNKS], f32)
            nc.sync.dma_start(mylsb[:], mylab_d.ap())
            iota_sb = cst.tile([128, C], f32)
            nc.sync.dma_start(iota_sb[:], iota_d.ap())
            ident_sb = cst.tile([128, 128], f32)
            nc.sync.dma_start(ident_sb[:], ident_d.ap())
            identb_sb = cst.tile([128, 128], bf16)
            nc.sync.dma_start(identb_sb[:], identb_d.ap())
            onesc_sb = cst.tile([128, 1], f32)
            nc.sync.dma_start(onesc_sb[:], onesc_d.ap())
            onesr_sb = cst.tile([1, 128], f32)
            nc.sync.dma_start(onesr_sb[:], onesr_d.ap())
            alpha_sb = cst.tile([128, 1], f32)
            nc.vector.memset(alpha_sb[:], ALPHA)

            # ---- preds: DMA groups split over the two HWDGE queues,
            #      bf16 copies chunk-pipelined on the vector engine ----
            psb = bigp.tile([128, CHUNKS, D], f32)
            psb_bf = bigp.tile([128, CHUNKS, D + 1], bf16)
            nc.vector.memset(psb_bf[:, :, D : D + 1], 1.0)
            preds_re = preds_d.ap().rearrange("(j p) d -> p j d", p=128)
            oh_all = bigp.tile([128, CHUNKS, C], bf16)
            iota_b = iota_sb[:].rearrange("p (j c) -> p j c", j=1)
            GROUPS = 8
            G = CHUNKS // GROUPS
            for g in range(GROUPS):
                eng = nc.sync if g % 2 == 0 else nc.scalar
                lo, hi = g * G, (g + 1) * G
                eng.dma_start(psb[:, lo:hi, :], preds_re[:, lo:hi, :])
                nc.vector.tensor_copy(
                    psb_bf[:, lo:hi, 0:D], psb[:, lo:hi, :]
                )
                # one-hot for this group's chunks (broadcast-AP equality)
                nc.vector.tensor_tensor(
                    oh_all[:, lo:hi, :],
                    lsb[:, lo:hi].to_broadcast((128, G, C)),
                    iota_b.to_broadcast((128, G, C)),
                    Alu.is_equal,
                )

            # own shard, chunk-major, fp32 + bf16
            osb = wrk.tile([128, OWN_CHUNKS, D], f32)
            nc.scalar.dma_start(
                osb[:], mypreds_d.ap().rearrange("(j p) d -> p j d", p=128)
            )
            osb_bf = wrk.tile([128, OWN_CHUNKS, D], bf16)
            nc.vector.tensor_copy(osb_bf[:], osb[:])

            # own-chunk masks: ohinv[:, j, 0, :] = 1e10*onehot (neg mask),
            #                  ohinv[:, j, 1, :] = 1e10*(1-onehot) (pos mask)
            mk = wrk.tile([128, OWN_CHUNKS, C], f32)
            nc.vector.tensor_tensor(
                mk[:],
                mylsb[:].to_broadcast((128, OWN_CHUNKS, C)),
                iota_b.to_broadcast((128, OWN_CHUNKS, C)),
                Alu.is_equal,
            )
            ohinv = wrk.tile([128, OWN_CHUNKS, 2, C], f32)
            nc.vector.tensor_scalar(
                ohinv[:, :, 0, :], mk[:], BIG, None, Alu.mult
            )
            nc.vector.tensor_scalar(
                ohinv[:, :, 1, :], mk[:], -BIG, BIG, Alu.mult, Alu.add
            )

            # ---- phase A: class sums + counts, even/odd col-packed ----
            # psum_cs2[c, :] (c<64): sums over even chunks for class c
            # psum_cs2[64+c, :]:     sums over odd chunks for class c
            psum_cs2 = pacc.tile([128, D + 1], f32)
            for j in range(CHUNKS):
                half = j % 2
                nc.tensor.matmul(
                    psum_cs2[64 * half : 64 * half + 64, :],
                    oh_all[:, j, :],
                    psb_bf[:, j, :],
                    start=(j < 2),
                    stop=(j >= CHUNKS - 2),
                    tile_position=(0, 64 * half),
                    skip_group_check=True,
                )

            # ---- own-shard prep (independent of phase A results) ----
            psq = wrk.tile([128, OWN_CHUNKS], f32)
            pts_bf = wrk.tile([128, OWN_CHUNKS, D], bf16)
            for j in range(OWN_CHUNKS):
                sqscr = scr.tile([128, D], f32, name=f"sqscr{j}", tag="sqscr")
                nc.scalar.activation(
                    sqscr[:], osb[:, j, :], Act.Square,
                    accum_out=psq[:, j : j + 1],
                )
                ptb = pt.tile([128, 128], bf16, name=f"ptb{j}", tag="ptb")
                nc.tensor.transpose(ptb[:], osb_bf[:, j, :], identb_sb[:])
                nc.scalar.activation(pts_bf[:, j, :], ptb[:], Act.Copy, scale=-2.0)

            # ---- centroids ----
            cs_sb = wrk.tile([128, D + 1], f32)
            nc.scalar.activation(cs_sb[:], psum_cs2[:], Act.Copy)
            # counts row [1, 128] (c2-indexed) via PE transpose of the column
            psum_cr = psm.tile([1, 128], f32, name="psum_cr", tag="sm")
            nc.tensor.matmul(psum_cr[:], cs_sb[:, D : D + 1], ident_sb[:])
            cr2 = wrk.tile([1, 128], f32)
            nc.scalar.activation(cr2[:], psum_cr[:], Act.Copy)
            cnt = wrk.tile([1, C], f32)
            nc.vector.tensor_tensor(
                cnt[:], cr2[:, 0:C], cr2[:, C : 2 * C], Alu.add
            )
            safe = wrk.tile([1, C], f32)
            nc.vector.tensor_scalar(safe[:], cnt[:], 1.0, None, Alu.max)
            rrow = wrk.tile([1, C], f32)
            nc.vector.reciprocal(rrow[:], safe[:])
            ab_sb = wrk.tile([1, C], f32)
            nc.vector.tensor_scalar(
                ab_sb[:], cnt[:], 0.0, HUGE, Alu.is_equal, Alu.mult
            )

            # centT[d, c] = (class_sum_even + class_sum_odd)[c, d] * recip[c]
            psum_ct = pt.tile([128, 128], f32, name="psum_ct", tag="ctp", bufs=1)
            nc.tensor.transpose(psum_ct[:], cs_sb[:, 0:D], ident_sb[:])
            ct_sb = wrk.tile([128, 128], f32)
            nc.scalar.activation(ct_sb[:], psum_ct[:], Act.Copy)
            ctsum = wrk.tile([128, C], f32)
            nc.vector.tensor_tensor(
                ctsum[:], ct_sb[:, 0:C], ct_sb[:, C : 2 * C], Alu.add
            )
            psum_rb = psm.tile([128, C], f32, name="psum_rb", tag="sm")
            nc.tensor.matmul(psum_rb[:], onesr_sb[:], rrow[:])
            centT = wrk.tile([128, C], f32)
            nc.vector.tensor_tensor(centT[:], ctsum[:], psum_rb[:], Alu.mult)
            centT_bf = wrk.tile([128, C], bf16)
            nc.vector.tensor_copy(centT_bf[:], centT[:])

            # c_sq row (+1e20 on absent classes), broadcast to all partitions
            sqc = wrk.tile([128, C], f32)
            nc.vector.tensor_tensor(sqc[:], centT[:], centT[:], Alu.mult)
            psum_csq = psm.tile([1, C], f32, name="psum_csq", tag="sm")
            nc.tensor.matmul(psum_csq[:], onesc_sb[:], sqc[:])
            csqr = wrk.tile([1, C], f32)
            nc.vector.tensor_tensor(csqr[:], psum_csq[:], ab_sb[:], Alu.add)
            psum_cb = psm.tile([128, C], f32, name="psum_cb", tag="sm")
            nc.tensor.matmul(psum_cb[:], onesr_sb[:], csqr[:])
            csq_sb = wrk.tile([128, C], f32)
            nc.scalar.activation(csq_sb[:], psum_cb[:], Act.Copy)

            # ---- phase F: per own chunk distances, masked mins ----
            # pnsq even cols = negsq (min over other classes), odd = possq
            pnsq = wrk.tile([128, 2 * OWN_CHUNKS], f32)
            for j in range(OWN_CHUNKS):
                psum_g = pg.tile([128, C], f32, name=f"psum_g{j}", tag="g")
                nc.tensor.matmul(psum_g[:], pts_bf[:, j, :], centT_bf[:])
                hc = scr.tile([128, C], f32, name=f"hc{j}", tag="hc")
                nc.vector.tensor_tensor(hc[:], psum_g[:], csq_sb[:], Alu.add)
                sqj = scr.tile([128, C], f32, name=f"sqj{j}", tag="sqj")
                nc.scalar.activation(
                    sqj[:], hc[:], Act.Relu, bias=psq[:, j : j + 1]
                )
                pair = scr.tile([128, 2, C], f32, name=f"pair{j}", tag="pair")
                nc.vector.tensor_tensor(
                    pair[:],
                    sqj[:].rearrange("p (u c) -> p u c", u=1).to_broadcast(
                        (128, 2, C)
                    ),
                    ohinv[:, j, :, :],
                    Alu.add,
                )
                nc.vector.tensor_reduce(
                    pnsq[:, 2 * j : 2 * j + 2], pair[:], Ax.X, Alu.min
                )

            # ---- tail: sqrt via Newton rsqrt on DVE, then softplus ----
            W = 2 * OWN_CHUNKS
            z = wrk.tile([128, W], f32)
            tsh = wrk.tile([128, W], f32)
            nc.vector.tensor_scalar(
                tsh[:].bitcast(i32), pnsq[:].bitcast(i32), 1, None,
                Alu.logical_shift_right,
            )
            nc.vector.tensor_scalar(
                z[:].bitcast(i32), tsh[:].bitcast(i32), -1, 0x5F3759DF,
                Alu.mult, Alu.add,
            )
            t1 = wrk.tile([128, W], f32)
            for _ in range(3):
                nc.vector.tensor_tensor(t1[:], z[:], z[:], Alu.mult)
                nc.vector.tensor_tensor(t1[:], t1[:], pnsq[:], Alu.mult)
                nc.vector.tensor_scalar(
                    t1[:], t1[:], -0.5, 1.5, Alu.mult, Alu.add
                )
                nc.vector.tensor_tensor(z[:], z[:], t1[:], Alu.mult)
            pn = wrk.tile([128, W], f32)
            nc.vector.tensor_tensor(pn[:], pnsq[:], z[:], Alu.mult)

            # softplus(pos - neg + alpha) = ln(1 + exp(...))
            x = wrk.tile([128, OWN_CHUNKS], f32)
            nc.vector.tensor_tensor(
                x[:], pn[:, 1::2], pn[:, 0::2], Alu.subtract
            )
            e = wrk.tile([128, OWN_CHUNKS], f32)
            nc.scalar.activation(e[:], x[:], Act.Exp, bias=alpha_sb[:])
            sp = wrk.tile([128, OWN_CHUNKS], f32)
            nc.scalar.activation(sp[:], e[:], Act.Ln, bias=1.0)
            rowsum = wrk.tile([128, 1], f32)
            nc.vector.tensor_reduce(rowsum[:], sp[:], Ax.X, Alu.add)
            psum_out = psm.tile([1, 1], f32, name="psum_out", tag="sm")
            nc.tensor.matmul(psum_out[:], rowsum[:], onesc_sb[:])
            out_sb = wrk.tile([1, 1], f32)
            nc.scalar.activation(out_sb[:], psum_out[:], Act.Copy)
            nc.sync.dma_start(out_d.ap(), out_sb[:])

    nc.compile()
    return nc


def _get_compiled():
    global _compiled
    if _compiled is None:
        _compiled = _build()
    return _compiled


def _chunk_major_labels(lab_f32):
    # labels[j*128 + p] -> [p, j]
    n_chunks = lab_f32.shape[0] // 128
    return np.ascontiguousarray(lab_f32.reshape(n_chunks, 128).T)


def kernel(preds, labels, _trace=False):
    preds = np.ascontiguousarray(np.asarray(preds, dtype=np.float32))
    lab_f32 = np.asarray(labels, dtype=np.float32)
    assert preds.shape == (N, D) and lab_f32.shape == (N,)

    nc = _get_compiled()
    lab_cm = _chunk_major_labels(lab_f32)
    in_maps = []
    for c in range(N_CORES):
        r0, r1 = c * ROWS_PER_CORE, (c + 1) * ROWS_PER_CORE
        in_maps.append(
            {
                "preds": preds,
                "labels": lab_cm,
                "my_preds": np.ascontiguousarray(preds[r0:r1]),
                "my_labels": _chunk_major_labels(lab_f32[r0:r1]),
            }
        )

    res = bass_utils.run_bass_kernel_spmd(
        nc, in_maps, core_ids=list(range(N_CORES)), trace=_trace
    )
    global last_results
    last_results = res
    total = sum(float(res.results[c]["out"][0, 0]) for c in range(N_CORES))
    return np.float32(total / N)


# revision 13
# speedup vs baseline: 1.3851x; 1.0467x over previous
"""Trainium2 Bass kernel for nn_CCL_50740743635433 (class-collapsed CCL loss).

Math: with C=64 classes, pos_centroid[i] == class_centroid[labels[i]], so the
reference's 8192x8192 distance matrix collapses to 8192x64:
  class_sum[c,:]  = sum_{i: lab_i==c} preds[i,:]      (one-hot matmul)
  cent[c,:]       = class_sum[c,:] / count[c]
  sq[i,c]         = relu(|p_i|^2 + |cent_c|^2 - 2 p_i.cent_c)
  pos[i]          = sqrt(sq[i, lab_i]);  neg[i] = sqrt(min_{c != lab_i} sq[i,c])
  loss            = mean softplus(pos - neg + 0.2)

Distribution (8 cores, no collectives): every core receives the FULL preds and
computes the class sums redundantly (a cross-core collective costs ~70us on
this rig vs ~12us of local compute); each core then evaluates distances +
softplus only for its own 1024-row shard and returns a partial sum; the host
adds the 8 partials and divides by N.

Perf structure (measured on this rig):
- Both big matmuls in bf16 (fp32 matmul is 4 cyc/row); verified numerically:
  the final loss moves ~3e-8 relative (errors wash out in the 8192-row mean).
- Phase A packs even/odd chunks into the two 64-column halves of the PE array
  (tile_position) so pairs of matmuls run concurrently; back-to-back matmuls
  pipeline at ~55ns each.
- Inputs stream in 8 DMA groups with per-group tiles (a single shared tile
  creates false WAW deps that serialize the DMAs), alternating between the
  two HWDGE queues (sync + scalar).
- f32->bf16 casts alternate between the vector and scalar engines (~0.7us /
  ~1.1us per 128x1k tile) so neither becomes the feed bottleneck.
- |cent_c|^2 is folded into the Gram PSUM by a K=1 rank-1 matmul, so the
  per-chunk distance needs only Relu(psum + p^2_bias) on the scalar engine.
- sqrt via 2-iteration Newton rsqrt on the vector engine keeps the scalar
  engine on one activation table (Copy/Square/Relu/Exp/Ln); a switch = ~2.7us.
"""

import sys

sys.path.insert(0, "/opt/trn_rl_repo")

import numpy as np

import concourse.bacc as bacc
import concourse.bass_utils as bass_utils
import concourse.mybir as mybir
import concourse.tile as tile

N = 8192
D = 128
C = 64
N_CORES = 8
ROWS_PER_CORE = N // N_CORES          # 1024
CHUNKS = N // 128                     # 64 chunks of 128 rows
OWN_CHUNKS = ROWS_PER_CORE // 128     # 8 chunks per core
GROUPS = 8
G = CHUNKS // GROUPS                  # chunks per DMA group
ALPHA = 0.2
BIG = 1e10
HUGE = 1e20

f32 = mybir.dt.float32
bf16 = mybir.dt.bfloat16
i32 = mybir.dt.int32
Alu = mybir.AluOpType
Act = mybir.ActivationFunctionType
Ax = mybir.AxisListType

_compiled = None
last_results = None


def _build():
    import ml_dtypes

    nc = bacc.Bacc(
        "TRN2",
        target_bir_lowering=False,
        debug=False,
        enable_asserts=True,
        num_devices=N_CORES,
    )

    preds_d = nc.dram_tensor("preds", [N, D], f32, kind="ExternalInput")
    labels_d = nc.dram_tensor("labels", [128, CHUNKS], f32, kind="ExternalInput")
    mypreds_d = nc.dram_tensor("my_preds", [ROWS_PER_CORE, D], f32, kind="ExternalInput")
    mylab_d = nc.dram_tensor("my_labels", [128, OWN_CHUNKS], f32, kind="ExternalInput")
    out_d = nc.dram_tensor("out", [1, 1], f32, kind="ExternalOutput")

    iota_d = nc.inline_tensor(
        np.tile(np.arange(C, dtype=np.float32), (128, 1)), name="iota64"
    )
    ident_d = nc.inline_tensor(np.eye(128, dtype=np.float32), name="ident128")
    identb_d = nc.inline_tensor(
        np.eye(128, dtype=ml_dtypes.bfloat16), name="identb128"
    )
    onesc_d = nc.inline_tensor(np.ones((128, 1), dtype=np.float32), name="ones_col")
    onesr_d = nc.inline_tensor(np.ones((1, 128), dtype=np.float32), name="ones_row")
    onesrb_d = nc.inline_tensor(
        np.ones((1, 128), dtype=ml_dtypes.bfloat16), name="onesrb"
    )

    with tile.TileContext(nc) as tc:
        with (
            tc.tile_pool(name="cst", bufs=1) as cst,
            tc.tile_pool(name="big", bufs=1) as bigp,
            tc.tile_pool(name="wrk", bufs=1) as wrk,
            tc.tile_pool(name="scr", bufs=2) as scr,
            tc.tile_pool(name="pacc", bufs=1, space="PSUM") as pacc,
            tc.tile_pool(name="pt", bufs=2, space="PSUM") as pt,
            tc.tile_pool(name="pg", bufs=2, space="PSUM") as pg,
            tc.tile_pool(name="psm", bufs=2, space="PSUM") as psm,
        ):
            # ---- small inputs first ----
            lsb = cst.tile([128, CHUNKS], f32)
            nc.sync.dma_start(lsb[:], labels_d.ap())
            mylsb = cst.tile([128, OWN_CHUNKS], f32)
            nc.sync.dma_start(mylsb[:], mylab_d.ap())
            iota_sb = cst.tile([128, C], f32)
            nc.sync.dma_start(iota_sb[:], iota_d.ap())
            iota_b = iota_sb[:].rearrange("p (j c) -> p j c", j=1)

            # one-hot per group, emitted early so the PE feed starts ASAP
            oh_g = []
            for g in range(GROUPS):
                t = bigp.tile([128, G, C], bf16, name=f"oh{g}", tag=f"oh{g}")
                nc.vector.tensor_tensor(
                    t[:],
                    lsb[:, g * G : (g + 1) * G].to_broadcast((128, G, C)),
                    iota_b.to_broadcast((128, G, C)),
                    Alu.is_equal,
                )
                oh_g.append(t)

            ident_sb = cst.tile([128, 128], f32)
            nc.sync.dma_start(ident_sb[:], ident_d.ap())
            identb_sb = cst.tile([128, 128], bf16)
            nc.sync.dma_start(identb_sb[:], identb_d.ap())
            onesc_sb = cst.tile([128, 1], f32)
            nc.sync.dma_start(onesc_sb[:], onesc_d.ap())
            onesr_sb = cst.tile([1, 128], f32)
            nc.sync.dma_start(onesr_sb[:], onesr_d.ap())
            onesrb_sb = cst.tile([1, 128], bf16)
            nc.sync.dma_start(onesrb_sb[:], onesrb_d.ap())
            alpha_sb = cst.tile([128, 1], f32)
            nc.vector.memset(alpha_sb[:], ALPHA)

            # ---- preds: per-group tiles, DMAs split over both HWDGE
            #      queues, casts alternating vector/scalar engines ----
            preds_re = preds_d.ap().rearrange("(j p) d -> p j d", p=128)
            psb_g, psbbf_g = [], []
            for g in range(GROUPS):
                pf = bigp.tile([128, G, D], f32, name=f"psb{g}", tag=f"psb{g}")
                pb = bigp.tile(
                    [128, G, D + 1], bf16, name=f"psbbf{g}", tag=f"psbbf{g}"
                )
                dma_eng = nc.sync if g % 2 == 0 else nc.scalar
                dma_eng.dma_start(pf[:], preds_re[:, g * G : (g + 1) * G, :])
                nc.vector.memset(pb[:, :, D : D + 1], 1.0)
                if g % 2 == 0:
                    nc.vector.tensor_copy(pb[:, :, 0:D], pf[:])
                else:
                    nc.scalar.activation(pb[:, :, 0:D], pf[:], Act.Copy)
                psb_g.append(pf)
                psbbf_g.append(pb)

            # own shard, chunk-major, fp32 + bf16
            osb = wrk.tile([128, OWN_CHUNKS, D], f32)
            nc.scalar.dma_start(
                osb[:], mypreds_d.ap().rearrange("(j p) d -> p j d", p=128)
            )
            osb_bf = wrk.tile([128, OWN_CHUNKS, D], bf16)
            nc.vector.tensor_copy(osb_bf[:], osb[:])

            # own-chunk masks: ohinv[:, j, 0, :] = 1e10*onehot (neg mask),
            #                  ohinv[:, j, 1, :] = 1e10*(1-onehot) (pos mask)
            mk = wrk.tile([128, OWN_CHUNKS, C], f32)
            nc.vector.tensor_tensor(
                mk[:],
                mylsb[:].to_broadcast((128, OWN_CHUNKS, C)),
                iota_b.to_broadcast((128, OWN_CHUNKS, C)),
                Alu.is_equal,
            )
            ohinv = wrk.tile([128, OWN_CHUNKS, 2, C], f32)
            nc.vector.tensor_scalar(
                ohinv[:, :, 0, :], mk[:], BIG, None, Alu.mult
            )
            nc.vector.tensor_scalar(
                ohinv[:, :, 1, :], mk[:], -BIG, BIG, Alu.mult, Alu.add
            )

            # ---- phase A: class sums + counts, even/odd col-packed ----
            # psum_cs2[c, :] (c<64): sums over even chunks for class c
            # psum_cs2[64+c, :]:     sums over odd chunks for class c
            psum_cs2 = pacc.tile([128, D + 1], f32)
            for j in range(CHUNKS):
                g, jj = j // G, j % G
                half = j % 2
                nc.tensor.matmul(
                    psum_cs2[64 * half : 64 * half + 64, :],
                    oh_g[g][:, jj, :],
                    psbbf_g[g][:, jj, :],
                    start=(j < 2),
                    stop=(j >= CHUNKS - 2),
                    tile_position=(0, 64 * half),
                    skip_group_check=True,
                )

            # ---- own-shard prep (independent of phase A results) ----
            psq = wrk.tile([128, OWN_CHUNKS], f32)
            pts_bf = wrk.tile([128, OWN_CHUNKS, D], bf16)
            for j in range(OWN_CHUNKS):
                sqscr = scr.tile([128, D], f32, name=f"sqscr{j}", tag="sqscr")
                nc.scalar.activation(
                    sqscr[:], osb[:, j, :], Act.Square,
                    accum_out=psq[:, j : j + 1],
                )
                ptb = pt.tile([128, 128], bf16, name=f"ptb{j}", tag="ptb")
                nc.tensor.transpose(ptb[:], osb_bf[:, j, :], identb_sb[:])
                nc.scalar.activation(pts_bf[:, j, :], ptb[:], Act.Copy, scale=-2.0)

            # ---- centroids ----
            cs_sb = wrk.tile([128, D + 1], f32)
            nc.scalar.activation(cs_sb[:], psum_cs2[:], Act.Copy)
            # counts row [1, 128] (c2-indexed) via PE transpose of the column
            psum_cr = psm.tile([1, 128], f32, name="psum_cr", tag="sm")
            nc.tensor.matmul(psum_cr[:], cs_sb[:, D : D + 1], ident_sb[:])
            cr2 = wrk.tile([1, 128], f32)
            nc.scalar.activation(cr2[:], psum_cr[:], Act.Copy)
            cnt = wrk.tile([1, C], f32)
            nc.vector.tensor_tensor(
                cnt[:], cr2[:, 0:C], cr2[:, C : 2 * C], Alu.add
            )
            safe = wrk.tile([1, C], f32)
            nc.vector.tensor_scalar(safe[:], cnt[:], 1.0, None, Alu.max)
            rrow = wrk.tile([1, C], f32)
            nc.vector.reciprocal(rrow[:], safe[:])
            ab_sb = wrk.tile([1, C], f32)
            nc.vector.tensor_scalar(
                ab_sb[:], cnt[:], 0.0, HUGE, Alu.is_equal, Alu.mult
            )

            # centT_bf[d, c] = (class_sum_even + class_sum_odd)[c,d] * recip[c]
            psum_ct = pt.tile([128, 128], f32, name="psum_ct", tag="ctp", bufs=1)
            nc.tensor.transpose(psum_ct[:], cs_sb[:, 0:D], ident_sb[:])
            ct_sb = wrk.tile([128, 128], f32)
            nc.scalar.activation(ct_sb[:], psum_ct[:], Act.Copy)
            ctsum = wrk.tile([128, C], f32)
            nc.vector.tensor_tensor(
                ctsum[:], ct_sb[:, 0:C], ct_sb[:, C : 2 * C], Alu.add
            )
            psum_rb = psm.tile([128, C], f32, name="psum_rb", tag="sm")
            nc.tensor.matmul(psum_rb[:], onesr_sb[:], rrow[:])
            centT_bf = wrk.tile([128, C], bf16)
            nc.vector.tensor_tensor(
                centT_bf[:], ctsum[:], psum_rb[:], Alu.mult
            )

            # c_sq row (+1e20 on absent classes) in bf16 for the rank-1 fold
            sqc = wrk.tile([128, C], f32)
            nc.vector.tensor_tensor(sqc[:], centT_bf[:], centT_bf[:], Alu.mult)
            psum_csq = psm.tile([1, C], f32, name="psum_csq", tag="sm")
            nc.tensor.matmul(psum_csq[:], onesc_sb[:], sqc[:])
            csqr_bf = wrk.tile([1, C], bf16)
            nc.vector.tensor_tensor(
                csqr_bf[:], psum_csq[:], ab_sb[:], Alu.add
            )

            # ---- phase F: per own chunk distances, masked mins ----
            # psum_g = -2*G + csq (rank-1 fold); sq = relu(psum_g + p^2)
            # pnsq even cols = negsq (min over other classes), odd = possq
            pnsq = wrk.tile([128, 2 * OWN_CHUNKS], f32)
            for j in range(OWN_CHUNKS):
                psum_g = pg.tile([128, C], f32, name=f"psum_g{j}", tag="g")
                nc.tensor.matmul(
                    psum_g[:], pts_bf[:, j, :], centT_bf[:],
                    start=True, stop=False,
                )
                nc.tensor.matmul(
                    psum_g[:], onesrb_sb[:], csqr_bf[:],
                    start=False, stop=True, skip_group_check=True,
                )
                sqj = scr.tile([128, C], f32, name=f"sqj{j}", tag="sqj")
                nc.scalar.activation(
                    sqj[:], psum_g[:], Act.Relu, bias=psq[:, j : j + 1]
                )
                pair = scr.tile([128, 2, C], f32, name=f"pair{j}", tag="pair")
                nc.vector.tensor_tensor(
                    pair[:],
                    sqj[:].rearrange("p (u c) -> p u c", u=1).to_broadcast(
                        (128, 2, C)
                    ),
                    ohinv[:, j, :, :],
                    Alu.add,
                )
                nc.vector.tensor_reduce(
                    pnsq[:, 2 * j : 2 * j + 2], pair[:], Ax.X, Alu.min
                )

            # ---- tail: sqrt via Newton rsqrt on DVE, then softplus ----
            W = 2 * OWN_CHUNKS
            z = wrk.tile([128, W], f32)
            tsh = wrk.tile([128, W], f32)
            nc.vector.tensor_scalar(
                tsh[:].bitcast(i32), pnsq[:].bitcast(i32), 1, None,
                Alu.logical_shift_right,
            )
            nc.vector.tensor_scalar(
                z[:].bitcast(i32), tsh[:].bitcast(i32), -1, 0x5F3759DF,
                Alu.mult, Alu.add,
            )
            t1 = wrk.tile([128, W], f32)
            for _ in range(2):
                nc.vector.tensor_tensor(t1[:], z[:], z[:], Alu.mult)
                nc.vector.tensor_tensor(t1[:], t1[:], pnsq[:], Alu.mult)
                nc.vector.tensor_scalar(
                    t1[:], t1[:], -0.5, 1.5, Alu.mult, Alu.add
                )
                nc.vector.tensor_tensor(z[:], z[:], t1[:], Alu.mult)
            pn = wrk.tile([128, W], f32)
            nc.vector.tensor_tensor(pn[:], pnsq[:], z[:], Alu.mult)

            # softplus(pos - neg + alpha) = ln(1 + exp(...))
            x = wrk.tile([128, OWN_CHUNKS], f32)
            nc.vector.tensor_tensor(
                x[:], pn[:, 1::2], pn[:, 0::2], Alu.subtract
            )
            e = wrk.tile([128, OWN_CHUNKS], f32)
            nc.scalar.activation(e[:], x[:], Act.Exp, bias=alpha_sb[:])
            sp = wrk.tile([128, OWN_CHUNKS], f32)
            nc.scalar.activation(sp[:], e[:], Act.Ln, bias=1.0)
            rowsum = wrk.tile([128, 1], f32)
            nc.vector.tensor_reduce(rowsum[:], sp[:], Ax.X, Alu.add)
            psum_out = psm.tile([1, 1], f32, name="psum_out", tag="sm")
            nc.tensor.matmul(psum_out[:], rowsum[:], onesc_sb[:])
            out_sb = wrk.tile([1, 1], f32)
            nc.scalar.activation(out_sb[:], psum_out[:], Act.Copy)
            nc.sync.dma_start(out_d.ap(), out_sb[:])

    nc.compile()
    return nc


def _get_compiled():
    global _compiled
    if _compiled is None:
        _compiled = _build()
    return _compiled


def _chunk_major_labels(lab_f32):
    # labels[j*128 + p] -> [p, j]
    n_chunks = lab_f32.shape[0] // 128
    return np.ascontiguousarray(lab_f32.reshape(n_chunks, 128).T)


def kernel(preds, labels, _trace=False):
    preds = np.ascontiguousarray(np.asarray(preds, dtype=np.float32))
    lab_f32 = np.asarray(labels, dtype=np.float32)
    assert preds.shape == (N, D) and lab_f32.shape == (N,)

    nc = _get_compiled()
    lab_cm = _chunk_major_labels(lab_f32)
    in_maps = []
    for c in range(N_CORES):
        r0, r1 = c * ROWS_PER_CORE, (c + 1) * ROWS_PER_CORE
        in_maps.append(
            {
                "preds": preds,
                "labels": lab_cm,
                "my_preds": np.ascontiguousarray(preds[r0:r1]),
                "my_labels": _chunk_major_labels(lab_f32[r0:r1]),
            }
        )

    res = bass_utils.run_bass_kernel_spmd(
        nc, in_maps, core_ids=list(range(N_CORES)), trace=_trace
    )
    global last_results
    last_results = res
    total = sum(float(res.results[c]["out"][0, 0]) for c in range(N_CORES))
    return np.float32(total / N)


# revision 15
# speedup vs baseline: 1.4254x; 1.0290x over previous
"""Trainium2 Bass kernel for nn_CCL_50740743635433 (class-collapsed CCL loss).

Math: with C=64 classes, pos_centroid[i] == class_centroid[labels[i]], so the
reference's 8192x8192 distance matrix collapses to 8192x64:
  class_sum[c,:]  = sum_{i: lab_i==c} preds[i,:]      (one-hot matmul)
  cent[c,:]       = class_sum[c,:] / count[c]
  sq[i,c]         = relu(|p_i|^2 + |cent_c|^2 - 2 p_i.cent_c)
  pos[i]          = sqrt(sq[i, lab_i]);  neg[i] = sqrt(min_{c != lab_i} sq[i,c])
  loss            = mean softplus(pos - neg + 0.2)

Distribution (8 cores, no collectives): every core receives the FULL preds and
computes the class sums redundantly (a cross-core collective costs ~70us on
this rig vs ~12us of local compute); each core then evaluates distances +
softplus only for its own 1024-row shard and returns a partial sum; the host
adds the 8 partials and divides by N.

Perf structure (measured on this rig):
- Both big matmuls in bf16 (fp32 matmul is 4 cyc/row); verified numerically:
  the final loss moves ~3e-8 relative (errors wash out in the 8192-row mean).
- Phase A packs even/odd chunks into the two 64-column halves of the PE array
  (tile_position) so pairs of matmuls run concurrently; back-to-back matmuls
  pipeline at ~55ns each.
- Inputs stream in 4 one-MB DMA groups with per-group tiles (shared tiles
  create false WAW deps that serialize DMAs) split across both HWDGE queues;
  constants are packed into two blob tensors to minimize DMA count.
- f32->bf16 casts alternate between vector and scalar engines per half-group.
- |cent_c|^2 is folded into the Gram PSUM by a K=1 rank-1 matmul, so the
  per-chunk distance needs only Relu(psum + p^2_bias) on the scalar engine.
- sqrt via 2-iteration Newton rsqrt on the vector engine; |p|^2 via
  tensor_scalar accum on the vector engine; the scalar engine then only uses
  Copy/Relu/Exp/Ln. Dummy Ln+Exp ops are emitted first so all activation
  table loads (~1.3us each) happen during the startup DMA window.
"""

import sys

sys.path.insert(0, "/opt/trn_rl_repo")

import numpy as np

import concourse.bacc as bacc
import concourse.bass_utils as bass_utils
import concourse.mybir as mybir
import concourse.tile as tile

N = 8192
D = 128
C = 64
N_CORES = 8
ROWS_PER_CORE = N // N_CORES          # 1024
CHUNKS = N // 128                     # 64 chunks of 128 rows
OWN_CHUNKS = ROWS_PER_CORE // 128     # 8 chunks per core
GROUPS = 4
G = CHUNKS // GROUPS                  # 16 chunks per DMA group
HALF = G // 2                         # cast granularity: 8 chunks
ALPHA = 0.2
BIG = 1e10
HUGE = 1e20

f32 = mybir.dt.float32
bf16 = mybir.dt.bfloat16
i32 = mybir.dt.int32
Alu = mybir.AluOpType
Act = mybir.ActivationFunctionType
Ax = mybir.AxisListType

_compiled = None
last_results = None


def _build():
    import ml_dtypes

    nc = bacc.Bacc(
        "TRN2",
        target_bir_lowering=False,
        debug=False,
        enable_asserts=True,
        num_devices=N_CORES,
    )

    preds_d = nc.dram_tensor("preds", [N, D], f32, kind="ExternalInput")
    labels_d = nc.dram_tensor("labels", [128, CHUNKS], f32, kind="ExternalInput")
    mypreds_d = nc.dram_tensor("my_preds", [ROWS_PER_CORE, D], f32, kind="ExternalInput")
    mylab_d = nc.dram_tensor("my_labels", [128, OWN_CHUNKS], f32, kind="ExternalInput")
    out_d = nc.dram_tensor("out", [1, 1], f32, kind="ExternalOutput")

    # constant blobs: one f32, one bf16 (fewer DMAs)
    # blob1 f32 [128, 321]: iota 0:64 | ident128 64:192 | ones_col 192:193 |
    #                       row0 of 193:321 = ones_row
    b1 = np.zeros((128, 321), dtype=np.float32)
    b1[:, 0:64] = np.arange(C, dtype=np.float32)[None, :]
    b1[:, 64:192] = np.eye(128, dtype=np.float32)
    b1[:, 192] = 1.0
    b1[0, 193:321] = 1.0
    blob1_d = nc.inline_tensor(b1, name="blob1")
    # blob2 bf16 [128, 256]: identb 0:128 | row0 of 128:256 = ones_row
    b2 = np.zeros((128, 256), dtype=ml_dtypes.bfloat16)
    b2[:, 0:128] = np.eye(128, dtype=ml_dtypes.bfloat16)
    b2[0, 128:256] = 1.0
    blob2_d = nc.inline_tensor(b2, name="blob2")

    with tile.TileContext(nc) as tc:
        with (
            tc.tile_pool(name="cst", bufs=1) as cst,
            tc.tile_pool(name="big", bufs=1) as bigp,
            tc.tile_pool(name="wrk", bufs=1) as wrk,
            tc.tile_pool(name="scr", bufs=2) as scr,
            tc.tile_pool(name="pacc", bufs=1, space="PSUM") as pacc,
            tc.tile_pool(name="pt", bufs=2, space="PSUM") as pt,
            tc.tile_pool(name="pg", bufs=2, space="PSUM") as pg,
            tc.tile_pool(name="psm", bufs=2, space="PSUM") as psm,
        ):
            # ---- small inputs / consts ----
            lsb = cst.tile([128, CHUNKS], f32)
            nc.sync.dma_start(lsb[:], labels_d.ap())
            mylsb = cst.tile([128, OWN_CHUNKS], f32)
            nc.sync.dma_start(mylsb[:], mylab_d.ap())
            blob1 = cst.tile([128, 321], f32)
            nc.sync.dma_start(blob1[:], blob1_d.ap())
            blob2 = cst.tile([128, 256], bf16)
            nc.sync.dma_start(blob2[:], blob2_d.ap())
            iota_sb = blob1[:, 0:64]
            ident_sb = blob1[:, 64:192]
            onesc_sb = blob1[:, 192:193]
            onesr_sb = blob1[0:1, 193:321]
            identb_sb = blob2[:, 0:128]
            onesrb_sb = blob2[0:1, 128:256]
            iota_b = iota_sb.rearrange("p (j c) -> p j c", j=1)
            alpha_sb = cst.tile([128, 1], f32)
            nc.vector.memset(alpha_sb[:], ALPHA)

            # dummy Ln+Exp first so activation-table loads happen at startup
            dmy = cst.tile([1, 1], f32)
            nc.scalar.activation(dmy[:], alpha_sb[0:1, :], Act.Ln, bias=1.0)
            nc.scalar.activation(dmy[:], dmy[:], Act.Exp, bias=alpha_sb[0:1, :])

            # own shard early (gates transposes/p^2 prep)
            osb = wrk.tile([128, OWN_CHUNKS, D], f32)
            nc.sync.dma_start(
                osb[:], mypreds_d.ap().rearrange("(j p) d -> p j d", p=128)
            )

            # one-hot per 8-chunk span, emitted early to start the PE feed
            NOH = CHUNKS // 8
            oh_g = []
            for q in range(NOH):
                t = bigp.tile([128, 8, C], bf16, name=f"oh{q}", tag=f"oh{q}")
                nc.vector.tensor_tensor(
                    t[:],
                    lsb[:, q * 8 : (q + 1) * 8].to_broadcast((128, 8, C)),
                    iota_b.to_broadcast((128, 8, C)),
                    Alu.is_equal,
                )
                oh_g.append(t)

            # ---- preds: 4 per-group tiles, DMAs on both HWDGE queues,
            #      casts per half-group alternating vector/scalar ----
            preds_re = preds_d.ap().rearrange("(j p) d -> p j d", p=128)
            psb_g, psbbf_g = [], []
            for g in range(GROUPS):
                pf = bigp.tile([128, G, D], f32, name=f"psb{g}", tag=f"psb{g}")
                pb = bigp.tile(
                    [128, G, D + 1], bf16, name=f"psbbf{g}", tag=f"psbbf{g}"
                )
                dma_eng = nc.sync if g < 2 else nc.scalar
                dma_eng.dma_start(pf[:], preds_re[:, g * G : (g + 1) * G, :])
                nc.vector.memset(pb[:, :, D : D + 1], 1.0)
                for h in range(2):
                    src = pf[:, h * HALF : (h + 1) * HALF, :]
                    dst = pb[:, h * HALF : (h + 1) * HALF, 0:D]
                    if (2 * g + h) % 2 == 0:
                        nc.vector.tensor_copy(dst, src)
                    else:
                        nc.scalar.activation(dst, src, Act.Copy)
                psb_g.append(pf)
                psbbf_g.append(pb)

            osb_bf = wrk.tile([128, OWN_CHUNKS, D], bf16)
            nc.vector.tensor_copy(osb_bf[:], osb[:])

            # own-chunk masks: ohinv[:, j, 0, :] = 1e10*onehot (neg mask),
            #                  ohinv[:, j, 1, :] = 1e10*(1-onehot) (pos mask)
            mk = wrk.tile([128, OWN_CHUNKS, C], f32)
            nc.vector.tensor_tensor(
                mk[:],
                mylsb[:].to_broadcast((128, OWN_CHUNKS, C)),
                iota_b.to_broadcast((128, OWN_CHUNKS, C)),
                Alu.is_equal,
            )
            ohinv = wrk.tile([128, OWN_CHUNKS, 2, C], f32)
            nc.vector.tensor_scalar(
                ohinv[:, :, 0, :], mk[:], BIG, None, Alu.mult
            )
            nc.vector.tensor_scalar(
                ohinv[:, :, 1, :], mk[:], -BIG, BIG, Alu.mult, Alu.add
            )

            # ---- phase A: class sums + counts, even/odd col-packed ----
            # psum_cs2[c, :] (c<64): sums over even chunks for class c
            # psum_cs2[64+c, :]:     sums over odd chunks for class c
            psum_cs2 = pacc.tile([128, D + 1], f32)
            for j in range(CHUNKS):
                g, jj = j // G, j % G
                q, qq = j // 8, j % 8
                half = j % 2
                nc.tensor.matmul(
                    psum_cs2[64 * half : 64 * half + 64, :],
                    oh_g[q][:, qq, :],
                    psbbf_g[g][:, jj, :],
                    start=(j < 2),
                    stop=(j >= CHUNKS - 2),
                    tile_position=(0, 64 * half),
                    skip_group_check=True,
                )

            # ---- own-shard prep (independent of phase A results) ----
            # p^2 via accumulating Square on the scalar engine, bf16
            # transposes on the PE, -2x copies on the scalar engine
            psq = wrk.tile([128, OWN_CHUNKS], f32)
            pts_bf = wrk.tile([128, OWN_CHUNKS, D], bf16)
            for j in range(OWN_CHUNKS):
                sqscr = scr.tile([128, D], f32, name=f"sqscr{j}", tag="sqscr")
                nc.scalar.activation(
                    sqscr[:], osb[:, j, :], Act.Square,
                    accum_out=psq[:, j : j + 1],
                )
                ptb = pt.tile([128, 128], bf16, name=f"ptb{j}", tag="ptb")
                nc.tensor.transpose(ptb[:], osb_bf[:, j, :], identb_sb)
                nc.scalar.activation(pts_bf[:, j, :], ptb[:], Act.Copy, scale=-2.0)

            # ---- centroids ----
            cs_sb = wrk.tile([128, D + 1], f32)
            nc.scalar.activation(cs_sb[:], psum_cs2[:], Act.Copy)
            # counts row [1, 128] (c2-indexed) via PE transpose of the column
            psum_cr = psm.tile([1, 128], f32, name="psum_cr", tag="sm")
            nc.tensor.matmul(psum_cr[:], cs_sb[:, D : D + 1], ident_sb)
            cr2 = wrk.tile([1, 128], f32)
            nc.scalar.activation(cr2[:], psum_cr[:], Act.Copy)
            cnt = wrk.tile([1, C], f32)
            nc.vector.tensor_tensor(
                cnt[:], cr2[:, 0:C], cr2[:, C : 2 * C], Alu.add
            )
            safe = wrk.tile([1, C], f32)
            nc.vector.tensor_scalar(safe[:], cnt[:], 1.0, None, Alu.max)
            rrow = wrk.tile([1, C], f32)
            nc.vector.reciprocal(rrow[:], safe[:])
            ab_sb = wrk.tile([1, C], f32)
            nc.vector.tensor_scalar(
                ab_sb[:], cnt[:], 0.0, HUGE, Alu.is_equal, Alu.mult
            )

            # centT_bf[d, c] = (class_sum_even + class_sum_odd)[c,d] * recip[c]
            psum_ct = pt.tile([128, 128], f32, name="psum_ct", tag="ctp", bufs=1)
            nc.tensor.transpose(psum_ct[:], cs_sb[:, 0:D], ident_sb)
            ct_sb = wrk.tile([128, 128], f32)
            nc.scalar.activation(ct_sb[:], psum_ct[:], Act.Copy)
            ctsum = wrk.tile([128, C], f32)
            nc.vector.tensor_tensor(
                ctsum[:], ct_sb[:, 0:C], ct_sb[:, C : 2 * C], Alu.add
            )
            psum_rb = psm.tile([128, C], f32, name="psum_rb", tag="sm")
            nc.tensor.matmul(psum_rb[:], onesr_sb, rrow[:])
            centT_bf = wrk.tile([128, C], bf16)
            nc.vector.tensor_tensor(
                centT_bf[:], ctsum[:], psum_rb[:], Alu.mult
            )

            # c_sq row (+1e20 on absent classes) in bf16 for the rank-1 fold
            sqc = wrk.tile([128, C], f32)
            nc.vector.tensor_tensor(sqc[:], centT_bf[:], centT_bf[:], Alu.mult)
            psum_csq = psm.tile([1, C], f32, name="psum_csq", tag="sm")
            nc.tensor.matmul(psum_csq[:], onesc_sb, sqc[:])
            csqr_bf = wrk.tile([1, C], bf16)
            nc.vector.tensor_tensor(
                csqr_bf[:], psum_csq[:], ab_sb[:], Alu.add
            )

            # ---- phase F: per own chunk distances, masked mins ----
            # psum_g = -2*G + csq (rank-1 fold); sq = relu(psum_g + p^2)
            # pnsq even cols = negsq (min over other classes), odd = possq
            pnsq = wrk.tile([128, 2 * OWN_CHUNKS], f32)
            for j in range(OWN_CHUNKS):
                psum_g = pg.tile([128, C], f32, name=f"psum_g{j}", tag="g")
                nc.tensor.matmul(
                    psum_g[:], pts_bf[:, j, :], centT_bf[:],
                    start=True, stop=False,
                )
                nc.tensor.matmul(
                    psum_g[:], onesrb_sb, csqr_bf[:],
                    start=False, stop=True, skip_group_check=True,
                )
                sqj = scr.tile([128, C], f32, name=f"sqj{j}", tag="sqj")
                nc.scalar.activation(
                    sqj[:], psum_g[:], Act.Relu, bias=psq[:, j : j + 1]
                )
                pair = scr.tile([128, 2, C], f32, name=f"pair{j}", tag="pair")
                nc.vector.tensor_tensor(
                    pair[:],
                    sqj[:].rearrange("p (u c) -> p u c", u=1).to_broadcast(
                        (128, 2, C)
                    ),
                    ohinv[:, j, :, :],
                    Alu.add,
                )
                nc.vector.tensor_reduce(
                    pnsq[:, 2 * j : 2 * j + 2], pair[:], Ax.X, Alu.min
                )

            # ---- tail: sqrt via Newton rsqrt on DVE, then softplus ----
            W = 2 * OWN_CHUNKS
            z = wrk.tile([128, W], f32)
            tsh = wrk.tile([128, W], f32)
            nc.vector.tensor_scalar(
                tsh[:].bitcast(i32), pnsq[:].bitcast(i32), 1, None,
                Alu.logical_shift_right,
            )
            nc.vector.tensor_scalar(
                z[:].bitcast(i32), tsh[:].bitcast(i32), -1, 0x5F3759DF,
                Alu.mult, Alu.add,
            )
            t1 = wrk.tile([128, W], f32)
            for _ in range(2):
                nc.vector.tensor_tensor(t1[:], z[:], z[:], Alu.mult)
                nc.vector.tensor_tensor(t1[:], t1[:], pnsq[:], Alu.mult)
                nc.vector.tensor_scalar(
                    t1[:], t1[:], -0.5, 1.5, Alu.mult, Alu.add
                )
                nc.vector.tensor_tensor(z[:], z[:], t1[:], Alu.mult)
            pn = wrk.tile([128, W], f32)
            nc.vector.tensor_tensor(pn[:], pnsq[:], z[:], Alu.mult)

            # softplus(pos - neg + alpha) = ln(1 + exp(...))
            x = wrk.tile([128, OWN_CHUNKS], f32)
            nc.vector.tensor_tensor(
                x[:], pn[:, 1::2], pn[:, 0::2], Alu.subtract
            )
            e = wrk.tile([128, OWN_CHUNKS], f32)
            nc.scalar.activation(e[:], x[:], Act.Exp, bias=alpha_sb[:])
            sp = wrk.tile([128, OWN_CHUNKS], f32)
            nc.scalar.activation(sp[:], e[:], Act.Ln, bias=1.0)
            rowsum = wrk.tile([128, 1], f32)
            nc.vector.tensor_reduce(rowsum[:], sp[:], Ax.X, Alu.add)
            psum_out = psm.tile([1, 1], f32, name="psum_out", tag="sm")
            nc.tensor.matmul(psum_out[:], rowsum[:], onesc_sb)
            out_sb = wrk.tile([1, 1], f32)
            nc.scalar.activation(out_sb[:], psum_out[:], Act.Copy)
            nc.sync.dma_start(out_d.ap(), out_sb[:])

    nc.compile()
    return nc


def _get_compiled():
    global _compiled
    if _compiled is None:
        _compiled = _build()
    return _compiled


def _chunk_major_labels(lab_f32):
    # labels[j*128 + p] -> [p, j]
    n_chunks = lab_f32.shape[0] // 128
    return np.ascontiguousarray(lab_f32.reshape(n_chunks, 128).T)


def kernel(preds, labels, _trace=False):
    preds = np.ascontiguousarray(np.asarray(preds, dtype=np.float32))
    lab_f32 = np.asarray(labels, dtype=np.float32)
    assert preds.shape == (N, D) and lab_f32.shape == (N,)

    nc = _get_compiled()
    lab_cm = _chunk_major_labels(lab_f32)
    in_maps = []
    for c in range(N_CORES):
        r0, r1 = c * ROWS_PER_CORE, (c + 1) * ROWS_PER_CORE
        in_maps.append(
            {
                "preds": preds,
                "labels": lab_cm,
                "my_preds": np.ascontiguousarray(preds[r0:r1]),
                "my_labels": _chunk_major_labels(lab_f32[r0:r1]),
            }
        )

    res = bass_utils.run_bass_kernel_spmd(
        nc, in_maps, core_ids=list(range(N_CORES)), trace=_trace
    )
    global last_results
    last_results = res
    total = sum(float(res.results[c]["out"][0, 0]) for c in range(N_CORES))
    return np.float32(total / N)
